# revision 9
# baseline (speedup 1.0000x reference)
"""KPConv Bass/Trainium2 kernel.

out[m,d] = sum_k ( sum_h infl[m,h,k] * s_feats[idx[m,h],:] ) @ W[k]
infl[m,h,k] = relu(1 - |s_pts[idx[m,h]] - q_pts[m] - kp[k]| / SIGMA)

Sharding: query points M=50000 split 8 ways (6250/core, padded to 6272 =
49 blocks x 128 points). Support table / weights / kernel_points
replicated per core.

Gather: support rows are packed into one u16 table [N, 256] (cols 0:128
= bf16 feats, 128:134 = f32 coords, rest pad; 512B rows). Per block of
128 query points (= 32 tiles of 4 points x 32 neighbors), 32 indirect
DMAs fetch one row per partition each (HW SWDGE semantics: one index
per partition, dest partition line = one table row) into comb
[128, 32*256] - edge (p, t) at u16 cols [t*256, (t+1)*256). Merging
feats+coords into one row halves the SWDGE instruction count vs
separate gathers; descriptor generation on the Q7 complex (~9ns/row)
is the kernel's critical path.

Per block: influence on DVE/ACT in f32 (delta, (delta-kp)^2, segmented
reduce, sqrt, relu affine); block-diag influence (bf16) on Pool; step A
on PE (bf16): per tile t, matmul(lhsT=feats_t [128e,128c], rhs=bd_t
[128e, 60]) -> PSUM wfT [128c, m*15+k]; step B on PE (bf16): per k,
matmul(lhsT=wfT[:, k::15], rhs=W[k]) accumulating -> [128m, 128d] ->
DRAM.
"""

import sys

sys.path.insert(0, "/opt/trn_rl_repo")

import numpy as np

# ---------------------------------------------------------------- constants
N_CORES = 8
M_TOTAL = 50000
N_SUP = 50000
H = 32
C = 128
K = 15
SIGMA = 2.0

M_CORE = M_TOTAL // N_CORES          # 6250
P = 128                              # partitions / points per block
NB = (M_CORE + P - 1) // P           # 49 blocks
M_PAD = NB * P                       # 6272
G = 4                                # points per step-A matmul tile
NT = P // G                          # 32 tiles per block
TW = 256                             # u16 cols per table row (512B)

_compiled = None


def _build_bass(nb=NB, n_sup=N_SUP, compile=True, repeats=1, parts="all"):
    """Build + compile the per-core SPMD Bass program."""
    from contextlib import ExitStack

    import concourse.bacc as bacc
    import concourse.mybir as mybir
    import concourse.tile as tile
    from concourse import bass

    f32 = mybir.dt.float32
    bf16 = mybir.dt.bfloat16
    u16 = mybir.dt.uint16
    i32 = mybir.dt.int32
    NB = nb
    N_SUP_ = n_sup

    nc = bacc.Bacc(
        "TRN2",
        target_bir_lowering=False,
        debug=False,
        enable_asserts=False,
        num_devices=N_CORES,
        dynamic_dma_scratch_size=65536,
    )

    q_blk_d = nc.dram_tensor("q_blk", (NB, P, NT * 3), f32, kind="ExternalInput")
    inds_d = nc.dram_tensor("inds_blk", (NB, P, NT), i32, kind="ExternalInput")
    table_d = nc.dram_tensor("table", (N_SUP_, TW), u16, kind="ExternalInput")
    w_d = nc.dram_tensor("w_ckd", (C, K * C), u16, kind="ExternalInput")
    kp_d = nc.dram_tensor("kp_rep", (P, K * 3), f32, kind="ExternalInput")
    mask_d = nc.dram_tensor("mask60", (P, G * K), f32, kind="ExternalInput")
    out_d = nc.dram_tensor("out", (NB, P, C), f32, kind="ExternalOutput")

    sub = mybir.AluOpType.subtract
    mult = mybir.AluOpType.mult

    with tile.TileContext(nc) as tc, ExitStack() as ctx:
        const = ctx.enter_context(tc.tile_pool(name="const", bufs=1))
        io = ctx.enter_context(tc.tile_pool(name="io", bufs=3))
        mid = ctx.enter_context(tc.tile_pool(name="mid", bufs=2))
        psa = ctx.enter_context(tc.tile_pool(name="psa", bufs=1, space="PSUM"))
        psb = ctx.enter_context(tc.tile_pool(name="psb", bufs=2, space="PSUM"))

        # constants: weights as [c, k*d] bf16, kernel points, block-diag mask
        w_sb = const.tile([P, K * C], u16)
        nc.sync.dma_start(w_sb[:], w_d.ap())
        w_view = w_sb[:].bitcast(bf16).rearrange("p (k d) -> p k d", d=C)
        kp_sb = const.tile([P, K * 3], f32)
        nc.sync.dma_start(kp_sb[:], kp_d.ap())
        mask_sb = const.tile([P, G * K], f32)
        nc.sync.dma_start(mask_sb[:], mask_d.ap())

        do_gather = parts in ("all", "gather")
        do_compute = parts in ("all", "compute")
        for B in [b for _ in range(repeats) for b in range(NB)]:
            inds = io.tile([P, NT], i32, tag="inds")
            nc.sync.dma_start(inds[:], inds_d.ap()[B])
            qb = io.tile([P, NT * 3], f32, tag="qb")
            nc.sync.dma_start(qb[:], q_blk_d.ap()[B])

            # gather: tile t -> u16 cols [t*TW, (t+1)*TW), one row/partition
            comb = io.tile([P, NT * TW], u16, tag="comb")
            if do_gather:
                for t in range(NT):
                    nc.gpsimd.indirect_dma_start(
                        out=comb[:, t * TW : (t + 1) * TW],
                        out_offset=None,
                        in_=table_d.ap(),
                        in_offset=bass.IndirectOffsetOnAxis(
                            ap=inds[:, t : t + 1], axis=0
                        ),
                    )
            else:
                nc.gpsimd.memset(comb[:], 0x3F80)
            if not do_compute:
                osb0 = mid.tile([P, C], f32, tag="osb")
                nc.vector.tensor_copy(osb0[:], comb[:, : 2 * C].bitcast(f32))
                nc.sync.dma_start(out_d.ap()[B], osb0[:])
                continue

            # influence (f32)
            combf = comb[:].bitcast(f32)                       # [P, NT*128]
            sgv = combf.rearrange("p (t x) -> p t x", x=TW // 2)[
                :, :, C // 2 : C // 2 + 3
            ]                                                   # [P, NT, 3]
            delta = mid.tile([P, NT * 3], f32, tag="delta")
            nc.vector.tensor_tensor(
                delta[:].rearrange("p (t j) -> p t j", j=3),
                sgv,
                qb[:].rearrange("p (t j) -> p t j", j=3),
                op=sub,
            )

            diff = mid.tile([P, NT * K * 3], f32, tag="diff")
            nc.vector.tensor_tensor(
                diff[:].rearrange("p (t k j) -> p t k j", k=K, j=3),
                delta[:].rearrange("p (t j) -> p t j", j=3)
                .unsqueeze(2)
                .broadcast_to([P, NT, K, 3]),
                kp_sb[:].rearrange("p (k j) -> p k j", j=3)
                .unsqueeze(1)
                .broadcast_to([P, NT, K, 3]),
                op=sub,
            )
            sq = mid.tile([P, NT * K * 3], f32, tag="sq")
            nc.vector.tensor_tensor(sq[:], diff[:], diff[:], op=mult)
            d2 = mid.tile([P, NT * K], f32, tag="d2")
            nc.vector.reduce_sum(
                out=d2[:],
                in_=sq[:].rearrange("p (tk j) -> p tk j", j=3),
                axis=mybir.AxisListType.X,
            )
            dd = mid.tile([P, NT * K], f32, tag="dd")
            nc.scalar.sqrt(dd[:], d2[:])
            infl = mid.tile([P, NT * K], f32, tag="infl")
            nc.scalar.activation(
                infl[:],
                dd[:],
                mybir.ActivationFunctionType.Relu,
                bias=1.0,
                scale=-1.0 / SIGMA,
            )

            # block-diagonal influence [p, t*60 + g*15 + k] (bf16, on DVE;
            # Pool is saturated with descriptor generation)
            bd = mid.tile([P, NT * G * K], bf16, tag="bd")
            nc.vector.tensor_tensor(
                bd[:].rearrange("p (t g k) -> p t g k", g=G, k=K),
                infl[:].rearrange("p (t k) -> p t k", k=K)
                .unsqueeze(2)
                .broadcast_to([P, NT, G, K]),
                mask_sb[:].rearrange("p (g k) -> p g k", k=K)
                .unsqueeze(1)
                .broadcast_to([P, NT, G, K]),
                op=mult,
            )

            # step A: 32 matmuls -> wfT[c, m*15+k] in 4 PSUM banks
            pa = [
                psa.tile([P, 8 * G * K], f32, tag=f"psA{q}", name=f"psA{q}")
                for q in range(4)
            ]
            for t in range(NT):
                nc.tensor.matmul(
                    pa[t // 8][:, (t % 8) * (G * K) : (t % 8 + 1) * (G * K)],
                    lhsT=comb[:, t * TW : t * TW + C].bitcast(bf16),
                    rhs=bd[:, t * (G * K) : (t + 1) * (G * K)],
                    start=True,
                    stop=True,
                )
            wfT = mid.tile([P, P * K], bf16, tag="wfT")
            for q in range(4):
                nc.scalar.copy(wfT[:, q * 480 : (q + 1) * 480], pa[q][:])

            # step B: accumulate over k
            outp = psb.tile([P, C], f32, tag="outp")
            wview = wfT[:].rearrange("p (m k) -> p k m", k=K)
            for k in range(K):
                nc.tensor.matmul(
                    outp[:],
                    lhsT=wview[:, k, :],
                    rhs=w_view[:, k, :],
                    start=(k == 0),
                    stop=(k == K - 1),
                )
            osb = mid.tile([P, C], f32, tag="osb")
            nc.scalar.copy(osb[:], outp[:])
            nc.sync.dma_start(out_d.ap()[B], osb[:])

    if compile:
        nc.compile()
    return nc


def _to_bf16_u16(x):
    """f32 ndarray -> bf16 bit pattern as u16 (round to nearest even)."""
    u = np.ascontiguousarray(x, np.float32).view(np.uint32)
    return ((u + 0x7FFF + ((u >> 16) & 1)) >> 16).astype(np.uint16)


def _host_prep(q_pts, s_pts, s_feats, neighb_inds, weights, kernel_points):
    """Shard + lay out inputs for the 8 cores."""
    q_pts = np.asarray(q_pts, np.float32)
    s_pts = np.asarray(s_pts, np.float32)
    s_feats = np.asarray(s_feats, np.float32)
    neighb_inds = np.asarray(neighb_inds, np.int64)
    weights = np.asarray(weights, np.float32)
    kernel_points = np.asarray(kernel_points, np.float32)

    # packed support table: bf16 feats + f32 coords per 512B row
    table = np.zeros((N_SUP, TW), np.uint16)
    table[:, :C] = _to_bf16_u16(s_feats)
    table[:, C : C + 6] = (
        np.ascontiguousarray(s_pts, "<f4").view(np.uint16).reshape(N_SUP, 6)
    )

    w_ckd = _to_bf16_u16(
        np.ascontiguousarray(weights.transpose(1, 0, 2)).reshape(C, K * C)
    )

    kp_rep = np.broadcast_to(
        kernel_points.reshape(1, K * 3), (P, K * 3)
    ).copy()
    mask60 = (
        (np.arange(G * K)[None, :] // K) == (np.arange(P)[:, None] // H)
    ).astype(np.float32)

    in_maps = []
    for i in range(N_CORES):
        sl = slice(i * M_CORE, (i + 1) * M_CORE)
        q = np.zeros((M_PAD, 3), np.float32)
        q[:M_CORE] = q_pts[sl]
        idx = np.zeros((M_PAD, H), np.int64)
        idx[:M_CORE] = neighb_inds[sl]

        # inds_blk[B, g*32+h, t] = idx[B*128 + 4t + g, h]
        a = idx.reshape(NB, NT, G, H)            # [B, t, g, h]
        inds_blk = np.ascontiguousarray(
            a.transpose(0, 2, 3, 1).astype(np.int32)  # [B, g, h, t]
        ).reshape(NB, P, NT)

        # q_blk[B, g*32+h, 3t+j] = q[B*128 + 4t + g, j]
        b = q.reshape(NB, NT, G, 3)              # [B, t, g, j]
        b = b.transpose(0, 2, 1, 3)              # [B, g, t, j]
        q_blk = np.repeat(
            b.reshape(NB, G, 1, NT * 3), H, axis=2
        ).reshape(NB, P, NT * 3)

        in_maps.append(
            {
                "q_blk": np.ascontiguousarray(q_blk),
                "inds_blk": inds_blk,
                "table": table,
                "w_ckd": w_ckd,
                "kp_rep": kp_rep,
                "mask60": mask60,
            }
        )
    return in_maps


def kernel(q_pts, s_pts, s_feats, neighb_inds, weights, kernel_points):
    global _compiled
    if _compiled is None:
        _compiled = _build_bass()
    nc = _compiled

    from concourse.bass_utils import run_bass_kernel_spmd

    in_maps = _host_prep(
        q_pts, s_pts, s_feats, neighb_inds, weights, kernel_points
    )
    res = run_bass_kernel_spmd(nc, in_maps, core_ids=list(range(N_CORES)))
    out = np.concatenate(
        [r["out"].reshape(M_PAD, C)[:M_CORE] for r in res.results], axis=0
    )
    return out.astype(np.float32)


if __name__ == "__main__":
    rng = np.random.default_rng(0)
    ins = {
        "q_pts": rng.standard_normal((M_TOTAL, 3), np.float32),
        "s_pts": rng.standard_normal((N_SUP, 3), np.float32),
        "s_feats": rng.standard_normal((N_SUP, 128), np.float32),
        "neighb_inds": rng.integers(0, N_SUP, (M_TOTAL, H)).astype(np.int32),
        "weights": rng.standard_normal((K, 128, 128), np.float32) * 0.05,
        "kernel_points": rng.standard_normal((K, 3), np.float32),
    }
    out = kernel(**ins)
    print(out.shape, out.dtype)


# revision 10
# speedup vs baseline: 1.0015x; 1.0015x over previous
"""KPConv Bass/Trainium2 kernel.

out[m,d] = sum_k ( sum_h infl[m,h,k] * s_feats[idx[m,h],:] ) @ W[k]
infl[m,h,k] = relu(1 - |s_pts[idx[m,h]] - q_pts[m] - kp[k]| / SIGMA)

Sharding: query points M=50000 split 8 ways (6250/core, padded to 6272 =
49 blocks x 128 points). Support table / weights / kernel_points
replicated per core.

Gather: support rows are packed into one u16 table [N, 256] (cols 0:128
= bf16 feats, 128:134 = f32 coords, rest pad; 512B rows). Per block of
128 query points (= 32 tiles of 4 points x 32 neighbors), 32 indirect
DMAs fetch one row per partition each (HW SWDGE semantics: one index
per partition, dest partition line = one table row) into comb
[128, 32*256] - edge (p, t) at u16 cols [t*256, (t+1)*256). Merging
feats+coords into one row halves the SWDGE instruction count vs
separate gathers; descriptor generation on the Q7 complex (~9ns/row)
is the kernel's critical path.

Per block: influence on DVE/ACT in f32 (delta, (delta-kp)^2, segmented
reduce, sqrt, relu affine); block-diag influence (bf16) on Pool; step A
on PE (bf16): per tile t, matmul(lhsT=feats_t [128e,128c], rhs=bd_t
[128e, 60]) -> PSUM wfT [128c, m*15+k]; step B on PE (bf16): per k,
matmul(lhsT=wfT[:, k::15], rhs=W[k]) accumulating -> [128m, 128d] ->
DRAM.
"""

import sys

sys.path.insert(0, "/opt/trn_rl_repo")

import numpy as np

# ---------------------------------------------------------------- constants
N_CORES = 8
M_TOTAL = 50000
N_SUP = 50000
H = 32
C = 128
K = 15
SIGMA = 2.0

M_CORE = M_TOTAL // N_CORES          # 6250
P = 128                              # partitions / points per block
NB = (M_CORE + P - 1) // P           # 49 blocks
M_PAD = NB * P                       # 6272
G = 4                                # points per step-A matmul tile
NT = P // G                          # 32 tiles per block
TW = 256                             # u16 cols per table row (512B)

_compiled = None


def _build_bass(nb=NB, n_sup=N_SUP, compile=True, repeats=1, parts="all"):
    """Build + compile the per-core SPMD Bass program."""
    from contextlib import ExitStack

    import concourse.bacc as bacc
    import concourse.mybir as mybir
    import concourse.tile as tile
    from concourse import bass

    f32 = mybir.dt.float32
    bf16 = mybir.dt.bfloat16
    u16 = mybir.dt.uint16
    i32 = mybir.dt.int32
    NB = nb
    N_SUP_ = n_sup

    nc = bacc.Bacc(
        "TRN2",
        target_bir_lowering=False,
        debug=False,
        enable_asserts=False,
        num_devices=N_CORES,
    )

    q_blk_d = nc.dram_tensor("q_blk", (NB, P, NT * 3), f32, kind="ExternalInput")
    inds_d = nc.dram_tensor("inds_blk", (NB, P, NT), i32, kind="ExternalInput")
    table_d = nc.dram_tensor("table", (N_SUP_, TW), u16, kind="ExternalInput")
    w_d = nc.dram_tensor("w_ckd", (C, K * C), u16, kind="ExternalInput")
    kp_d = nc.dram_tensor("kp_rep", (P, K * 3), f32, kind="ExternalInput")
    mask_d = nc.dram_tensor("mask60", (P, G * K), f32, kind="ExternalInput")
    out_d = nc.dram_tensor("out", (NB, P, C), f32, kind="ExternalOutput")

    sub = mybir.AluOpType.subtract
    mult = mybir.AluOpType.mult

    with tile.TileContext(nc) as tc, ExitStack() as ctx:
        const = ctx.enter_context(tc.tile_pool(name="const", bufs=1))
        io = ctx.enter_context(tc.tile_pool(name="io", bufs=3))
        mid = ctx.enter_context(tc.tile_pool(name="mid", bufs=2))
        psa = ctx.enter_context(tc.tile_pool(name="psa", bufs=1, space="PSUM"))
        psb = ctx.enter_context(tc.tile_pool(name="psb", bufs=2, space="PSUM"))

        # constants: weights as [c, k*d] bf16, kernel points, block-diag mask
        w_sb = const.tile([P, K * C], u16)
        nc.sync.dma_start(w_sb[:], w_d.ap())
        w_view = w_sb[:].bitcast(bf16).rearrange("p (k d) -> p k d", d=C)
        kp_sb = const.tile([P, K * 3], f32)
        nc.sync.dma_start(kp_sb[:], kp_d.ap())
        mask_sb = const.tile([P, G * K], f32)
        nc.sync.dma_start(mask_sb[:], mask_d.ap())

        do_gather = parts in ("all", "gather")
        do_compute = parts in ("all", "compute")
        for B in [b for _ in range(repeats) for b in range(NB)]:
            inds = io.tile([P, NT], i32, tag="inds")
            nc.sync.dma_start(inds[:], inds_d.ap()[B])
            qb = io.tile([P, NT * 3], f32, tag="qb")
            nc.sync.dma_start(qb[:], q_blk_d.ap()[B])

            # gather: tile t -> u16 cols [t*TW, (t+1)*TW), one row/partition
            comb = io.tile([P, NT * TW], u16, tag="comb")
            if do_gather:
                for t in range(NT):
                    nc.gpsimd.indirect_dma_start(
                        out=comb[:, t * TW : (t + 1) * TW],
                        out_offset=None,
                        in_=table_d.ap(),
                        in_offset=bass.IndirectOffsetOnAxis(
                            ap=inds[:, t : t + 1], axis=0
                        ),
                    )
            else:
                nc.gpsimd.memset(comb[:], 0x3F80)
            if not do_compute:
                osb0 = mid.tile([P, C], f32, tag="osb")
                nc.vector.tensor_copy(osb0[:], comb[:, : 2 * C].bitcast(f32))
                nc.sync.dma_start(out_d.ap()[B], osb0[:])
                continue

            # influence (f32)
            combf = comb[:].bitcast(f32)                       # [P, NT*128]
            sgv = combf.rearrange("p (t x) -> p t x", x=TW // 2)[
                :, :, C // 2 : C // 2 + 3
            ]                                                   # [P, NT, 3]
            delta = mid.tile([P, NT * 3], f32, tag="delta")
            nc.vector.tensor_tensor(
                delta[:].rearrange("p (t j) -> p t j", j=3),
                sgv,
                qb[:].rearrange("p (t j) -> p t j", j=3),
                op=sub,
            )

            diff = mid.tile([P, NT * K * 3], f32, tag="diff")
            nc.vector.tensor_tensor(
                diff[:].rearrange("p (t k j) -> p t k j", k=K, j=3),
                delta[:].rearrange("p (t j) -> p t j", j=3)
                .unsqueeze(2)
                .broadcast_to([P, NT, K, 3]),
                kp_sb[:].rearrange("p (k j) -> p k j", j=3)
                .unsqueeze(1)
                .broadcast_to([P, NT, K, 3]),
                op=sub,
            )
            sq = mid.tile([P, NT * K * 3], f32, tag="sq")
            nc.vector.tensor_tensor(sq[:], diff[:], diff[:], op=mult)
            d2 = mid.tile([P, NT * K], f32, tag="d2")
            nc.vector.reduce_sum(
                out=d2[:],
                in_=sq[:].rearrange("p (tk j) -> p tk j", j=3),
                axis=mybir.AxisListType.X,
            )
            dd = mid.tile([P, NT * K], f32, tag="dd")
            nc.scalar.sqrt(dd[:], d2[:])
            infl = mid.tile([P, NT * K], f32, tag="infl")
            nc.scalar.activation(
                infl[:],
                dd[:],
                mybir.ActivationFunctionType.Relu,
                bias=1.0,
                scale=-1.0 / SIGMA,
            )

            # block-diagonal influence [p, t*60 + g*15 + k] (bf16, on DVE;
            # Pool is saturated with descriptor generation)
            bd = mid.tile([P, NT * G * K], bf16, tag="bd")
            nc.vector.tensor_tensor(
                bd[:].rearrange("p (t g k) -> p t g k", g=G, k=K),
                infl[:].rearrange("p (t k) -> p t k", k=K)
                .unsqueeze(2)
                .broadcast_to([P, NT, G, K]),
                mask_sb[:].rearrange("p (g k) -> p g k", k=K)
                .unsqueeze(1)
                .broadcast_to([P, NT, G, K]),
                op=mult,
            )

            # step A: 32 matmuls -> wfT[c, m*15+k] in 4 PSUM banks
            pa = [
                psa.tile([P, 8 * G * K], f32, tag=f"psA{q}", name=f"psA{q}")
                for q in range(4)
            ]
            for t in range(NT):
                nc.tensor.matmul(
                    pa[t // 8][:, (t % 8) * (G * K) : (t % 8 + 1) * (G * K)],
                    lhsT=comb[:, t * TW : t * TW + C].bitcast(bf16),
                    rhs=bd[:, t * (G * K) : (t + 1) * (G * K)],
                    start=True,
                    stop=True,
                )
            wfT = mid.tile([P, P * K], bf16, tag="wfT")
            for q in range(4):
                nc.scalar.copy(wfT[:, q * 480 : (q + 1) * 480], pa[q][:])

            # step B: accumulate over k
            outp = psb.tile([P, C], f32, tag="outp")
            wview = wfT[:].rearrange("p (m k) -> p k m", k=K)
            for k in range(K):
                nc.tensor.matmul(
                    outp[:],
                    lhsT=wview[:, k, :],
                    rhs=w_view[:, k, :],
                    start=(k == 0),
                    stop=(k == K - 1),
                )
            osb = mid.tile([P, C], f32, tag="osb")
            nc.scalar.copy(osb[:], outp[:])
            nc.sync.dma_start(out_d.ap()[B], osb[:])

    if compile:
        nc.compile()
    return nc


def _to_bf16_u16(x):
    """f32 ndarray -> bf16 bit pattern as u16 (round to nearest even)."""
    u = np.ascontiguousarray(x, np.float32).view(np.uint32)
    return ((u + 0x7FFF + ((u >> 16) & 1)) >> 16).astype(np.uint16)


def _host_prep(q_pts, s_pts, s_feats, neighb_inds, weights, kernel_points):
    """Shard + lay out inputs for the 8 cores."""
    q_pts = np.asarray(q_pts, np.float32)
    s_pts = np.asarray(s_pts, np.float32)
    s_feats = np.asarray(s_feats, np.float32)
    neighb_inds = np.asarray(neighb_inds, np.int64)
    weights = np.asarray(weights, np.float32)
    kernel_points = np.asarray(kernel_points, np.float32)

    # packed support table: bf16 feats + f32 coords per 512B row
    table = np.zeros((N_SUP, TW), np.uint16)
    table[:, :C] = _to_bf16_u16(s_feats)
    table[:, C : C + 6] = (
        np.ascontiguousarray(s_pts, "<f4").view(np.uint16).reshape(N_SUP, 6)
    )

    w_ckd = _to_bf16_u16(
        np.ascontiguousarray(weights.transpose(1, 0, 2)).reshape(C, K * C)
    )

    kp_rep = np.broadcast_to(
        kernel_points.reshape(1, K * 3), (P, K * 3)
    ).copy()
    mask60 = (
        (np.arange(G * K)[None, :] // K) == (np.arange(P)[:, None] // H)
    ).astype(np.float32)

    in_maps = []
    for i in range(N_CORES):
        sl = slice(i * M_CORE, (i + 1) * M_CORE)
        q = np.zeros((M_PAD, 3), np.float32)
        q[:M_CORE] = q_pts[sl]
        idx = np.zeros((M_PAD, H), np.int64)
        idx[:M_CORE] = neighb_inds[sl]

        # inds_blk[B, g*32+h, t] = idx[B*128 + 4t + g, h]
        a = idx.reshape(NB, NT, G, H)            # [B, t, g, h]
        inds_blk = np.ascontiguousarray(
            a.transpose(0, 2, 3, 1).astype(np.int32)  # [B, g, h, t]
        ).reshape(NB, P, NT)

        # q_blk[B, g*32+h, 3t+j] = q[B*128 + 4t + g, j]
        b = q.reshape(NB, NT, G, 3)              # [B, t, g, j]
        b = b.transpose(0, 2, 1, 3)              # [B, g, t, j]
        q_blk = np.repeat(
            b.reshape(NB, G, 1, NT * 3), H, axis=2
        ).reshape(NB, P, NT * 3)

        in_maps.append(
            {
                "q_blk": np.ascontiguousarray(q_blk),
                "inds_blk": inds_blk,
                "table": table,
                "w_ckd": w_ckd,
                "kp_rep": kp_rep,
                "mask60": mask60,
            }
        )
    return in_maps


def kernel(q_pts, s_pts, s_feats, neighb_inds, weights, kernel_points):
    global _compiled
    if _compiled is None:
        _compiled = _build_bass()
    nc = _compiled

    from concourse.bass_utils import run_bass_kernel_spmd

    in_maps = _host_prep(
        q_pts, s_pts, s_feats, neighb_inds, weights, kernel_points
    )
    res = run_bass_kernel_spmd(nc, in_maps, core_ids=list(range(N_CORES)))
    out = np.concatenate(
        [r["out"].reshape(M_PAD, C)[:M_CORE] for r in res.results], axis=0
    )
    return out.astype(np.float32)


if __name__ == "__main__":
    rng = np.random.default_rng(0)
    ins = {
        "q_pts": rng.standard_normal((M_TOTAL, 3), np.float32),
        "s_pts": rng.standard_normal((N_SUP, 3), np.float32),
        "s_feats": rng.standard_normal((N_SUP, 128), np.float32),
        "neighb_inds": rng.integers(0, N_SUP, (M_TOTAL, H)).astype(np.int32),
        "weights": rng.standard_normal((K, 128, 128), np.float32) * 0.05,
        "kernel_points": rng.standard_normal((K, 3), np.float32),
    }
    out = kernel(**ins)
    print(out.shape, out.dtype)
